# revision 27
# baseline (speedup 1.0000x reference)
"""MoE FFN (top-2 of 8 experts) Trainium2 kernel, pair/F-split variant.

Same host-side routing as the resident-weight kernel, but load-balanced:
experts are sorted by token count and paired heavy-with-light; pair p is
assigned to cores (2p, 2p+1).  Each core of the pair holds HALF the F
dimension of BOTH experts' weights (same 16.8 MB bf16 SBUF footprint as
one full expert) and processes ALL of the pair's tokens at half F:

    per-core slots = ca_pad + cb_pad  (~4352 half-F token slots)
                   = ~2176 full-token equivalents vs 2304 for pure
                     expert-parallel -> ~6% less PE work.

GEMM2 contracts only the local F half, so each token's output is a
partial; the host sums the two cores' partials ("combine" still on
host).  Everything else (bf16, resident weights, chunk-major x, lag-1
GEMM2 pipeline, graded weight-slice streaming) matches the resident
kernel.
"""

import os
import sys
import numpy as np

for _p in ("/opt/trn_rl_repo", "/root/.axon_site/_ro/trn_rl_repo"):
    if _p not in sys.path and os.path.isdir(_p):
        sys.path.append(_p)

import concourse.bacc as bacc  # noqa: E402
import concourse.tile as tile  # noqa: E402
from concourse import mybir  # noqa: E402
from concourse.bass_utils import run_bass_kernel_spmd  # noqa: E402

# Problem shapes (hardcoded per spec)
B, S, H, F, E = 4, 2048, 1024, 4096, 8
T = B * S
TOP_K = 2
N_CORES = 8
P = 128
KH = H // P          # 8   H-contraction subtiles
F2 = F // 2          # 2048 F per core (half of one expert)
FT2 = F2 // P        # 16  f-tiles per expert half
CHUNK = 256          # tokens per GEMM1 chunk

F32 = mybir.dt.float32
BF16 = mybir.dt.bfloat16

_CACHE: dict = {}
LAST_RESULT = None  # BassKernelResults of the most recent run (for test.py)


def _build(ca: int, cb: int, use_b1: bool, mm_dt):
    nc = bacc.Bacc(
        "TRN2",
        target_bir_lowering=False,
        debug=False,
        enable_asserts=False,
        num_devices=N_CORES,
    )

    # ca/cb are EXACT max token counts (ceil-8); GEMM1 runs only the real
    # tokens while GEMM2/yd cover ceil-128 rows (t-tile granularity). The
    # few G2 rows beyond the exact count read stale hq data; their outputs
    # are padding rows the host discards, and a matmul's token columns are
    # independent so no real row is contaminated.
    ca_pad = -(-ca // P) * P
    cb_pad = -(-cb // P) * P
    c_pad = ca_pad + cb_pad
    NF = 2 * FT2  # 32 f-tiles total (16 per expert)
    # mixed 512/tail token chunks (512 keeps GEMM1 instruction count low)
    chunks = []  # (token_offset, nt, f0)
    for base, ce, f0 in ((0, ca, 0), (ca_pad, cb, FT2)):
        off = base
        sizes = [512] * (ce // 512)
        if ce % 512:
            sizes.append(ce % 512)
        for nt in sizes:
            chunks.append((off, nt, f0))
            off += nt
    n_chunks = len(chunks)

    # flat x layout: each chunk is a contiguous [KH, nt] block per partition
    xd = nc.dram_tensor(
        "xd", [P, KH * c_pad], mm_dt, kind="ExternalInput"
    ).ap()
    w1d = nc.dram_tensor("w1d", [P, NF, KH, P], mm_dt, kind="ExternalInput").ap()
    w2d = nc.dram_tensor("w2d", [P, NF, H], mm_dt, kind="ExternalInput").ap()
    if use_b1:
        b1d = nc.dram_tensor("b1d", [P, NF], F32, kind="ExternalInput").ap()
    yd = nc.dram_tensor("yd", [P, c_pad // P, H], F32, kind="ExternalOutput").ap()

    gelu = mybir.ActivationFunctionType.Gelu_apprx_tanh

    with tile.TileContext(nc) as tc:
        with (
            tc.tile_pool(name="w1p", bufs=1) as w1p,
            tc.tile_pool(name="w2p", bufs=1) as w2p,
            tc.tile_pool(name="xp", bufs=2) as xp,
            tc.tile_pool(name="hp", bufs=3) as hp,
            tc.tile_pool(name="op", bufs=4) as op,
            tc.tile_pool(name="bp", bufs=1) as bp,
            tc.tile_pool(name="ps1", bufs=3, space="PSUM") as ps1,
            tc.tile_pool(name="ps2", bufs=4, space="PSUM") as ps2,
            tc.tile_pool(name="wup", bufs=1) as wup,
            tc.tile_pool(name="psw", bufs=1, space="PSUM") as psw,
        ):
            # PE warmup: dummy matmuls on a zeroed scratch tile fill the
            # initial DMA-wait window so the HAM clock gate reaches 8/8
            # (2.4 GHz) before the first real matmul (~3.4us of sustained
            # PE activity required).
            wt = wup.tile([P, 512], mm_dt)
            nc.gpsimd.memset(wt[:], 0.0)
            wu_ps = psw.tile([P, 512], F32)
            for _ in range(26):
                nc.tensor.matmul(wu_ps[:], wt[:, :P], wt[:], start=True, stop=True)

            if use_b1:
                b1t = bp.tile([P, NF], F32)
                nc.sync.dma_start(b1t[:], b1d[:])

            # weight stream order: w1(expert a) graded, w2(a), w1(b), w2(b)
            w1 = w1p.tile([P, NF, KH, P], mm_dt)
            w2 = w2p.tile([P, NF, H], mm_dt)

            def stream_w(eh):  # eh = 0 (expert a) or 1 (expert b)
                base = eh * FT2
                slices = (1, 1, 2, 4, 8) if eh == 0 else (8, 8)
                i = base
                for n in slices:
                    nc.scalar.dma_start(w1[:, i : i + n], w1d[:, i : i + n])
                    i += n
                for i in range(base, base + FT2, 8):
                    nc.scalar.dma_start(w2[:, i : i + 8], w2d[:, i : i + 8])

            stream_w(0)
            stream_w(1)

            hqs = [None] * n_chunks

            def gemm1(ci):
                off, nt, f0 = chunks[ci]
                xt = xp.tile([P, KH * 512], mm_dt, tag="xt", name=f"xt_{ci}")
                nc.sync.dma_start(
                    xt[:, : KH * nt], xd[:, KH * off : KH * (off + nt)]
                )
                hq = hp.tile([P, FT2, 512], mm_dt, tag="hq", name=f"hq_{ci}")
                hqs[ci] = hq
                for fi in range(FT2):
                    f = f0 + fi
                    pt1 = ps1.tile([P, 512], F32, tag="pt1")
                    for k in range(KH):
                        nc.tensor.matmul(
                            pt1[:, :nt],
                            w1[:, f, k, :],
                            xt[:, k * nt : (k + 1) * nt],
                            start=(k == 0),
                            stop=(k == KH - 1),
                        )
                    bias = b1t[:, f : f + 1] if use_b1 else 0.0
                    nc.scalar.activation(hq[:, fi, :nt], pt1[:, :nt], gelu, bias=bias)

            def gemm2(ci):
                off, nt, f0 = chunks[ci]
                hq = hqs[ci]
                tcount = -(-nt // P)
                for t in range(tcount):
                    trow = off // P + t
                    last = ci == n_chunks - 1 and t == tcount - 1
                    pts = [
                        ps2.tile([P, 512], F32, tag="pt2", name=f"pt2_{hh}")
                        for hh in range(2)
                    ]
                    # last tile: finish hh halves sequentially so the first
                    # output DMA overlaps the second half's matmuls
                    hh_groups = [[0], [1]] if last else [[0, 1]]
                    for hhg in hh_groups:
                        for k2 in range(FT2):
                            for hh in hhg:
                                nc.tensor.matmul(
                                    pts[hh][:],
                                    hq[:, k2, t * P : (t + 1) * P],
                                    w2[:, f0 + k2, hh * 512 : (hh + 1) * 512],
                                    start=(k2 == 0),
                                    stop=(k2 == FT2 - 1),
                                )
                        if last:
                            hh = hhg[0]
                            # 2 strips: the final 128KB DMA finishes sooner
                            for q in range(2):
                                ot = op.tile([P, 512], F32, tag="ot")
                                nc.vector.tensor_copy(
                                    ot[:, :256], pts[hh][:, q * 256 : (q + 1) * 256]
                                )
                                nc.sync.dma_start(
                                    yd[
                                        :,
                                        trow,
                                        hh * 512 + q * 256 : hh * 512 + (q + 1) * 256,
                                    ],
                                    ot[:, :256],
                                )
                    if last:
                        hqs[ci] = None
                        return
                    for hh in range(2):
                        ot = op.tile([P, 512], F32, tag="ot")
                        nc.vector.tensor_copy(ot[:], pts[hh][:])
                        nc.sync.dma_start(
                            yd[:, trow, hh * 512 : (hh + 1) * 512], ot[:]
                        )
                hqs[ci] = None

            for ci in range(n_chunks):
                gemm1(ci)
                if ci >= 1:
                    gemm2(ci - 1)
            gemm2(n_chunks - 1)

    nc.compile()
    return nc


def _route(x2d, Wg):
    """Replicates reference router: softmax -> top-2 -> renormalize."""
    logits = x2d @ Wg  # [T, E] fp32
    m = logits.max(axis=-1, keepdims=True)
    p = np.exp(logits - m, dtype=np.float32)
    p /= p.sum(axis=-1, keepdims=True)
    # jax.lax.top_k: values descending, ties broken by lower index.
    order = np.argsort(-p, axis=-1, kind="stable")
    top_i = order[:, :TOP_K]  # [T, 2]
    top_p = np.take_along_axis(p, top_i, axis=-1)
    top_p = top_p / top_p.sum(axis=-1, keepdims=True)
    return top_i, top_p


def _pad8(c: int) -> int:
    return max(8, ((c + 7) // 8) * 8)


def kernel(x, Wg, W1, b1, W2, b2):
    global LAST_RESULT
    x = np.ascontiguousarray(np.asarray(x, dtype=np.float32))
    Wg = np.ascontiguousarray(np.asarray(Wg, dtype=np.float32))
    W1 = np.ascontiguousarray(np.asarray(W1, dtype=np.float32))
    b1 = np.ascontiguousarray(np.asarray(b1, dtype=np.float32))
    W2 = np.ascontiguousarray(np.asarray(W2, dtype=np.float32))
    b2 = np.ascontiguousarray(np.asarray(b2, dtype=np.float32))

    x2d = x.reshape(T, H)
    top_i, top_p = _route(x2d, Wg)

    rows = [None] * E
    gval = [None] * E
    for e in range(E):
        r, slot = np.nonzero(top_i == e)
        rows[e] = r
        gval[e] = top_p[r, slot]

    counts = np.array([len(r) for r in rows])
    order = np.argsort(-counts, kind="stable")
    pairs = [(int(order[i]), int(order[E - 1 - i])) for i in range(E // 2)]
    ca = _pad8(int(counts[[p[0] for p in pairs]].max()))
    cb = _pad8(int(counts[[p[1] for p in pairs]].max()))
    use_b1 = bool(np.any(b1))

    mm_dt = {
        "bf16": BF16,
        "fp32": F32,
    }[os.environ.get("KERNEL_MMDT", "bf16")]
    key = (ca, cb, use_b1, str(mm_dt))
    if key not in _CACHE:
        _CACHE[key] = _build(ca, cb, use_b1, mm_dt)
    nc = _CACHE[key]

    np_dt = mybir.dt.np(mm_dt)
    ca_pad = -(-ca // P) * P
    cb_pad = -(-cb // P) * P
    c_pad = ca_pad + cb_pad
    host_chunks = []  # (token_offset, nt) mirroring _build
    for base, ce in ((0, ca), (ca_pad, cb)):
        off = base
        sizes = [512] * (ce // 512)
        if ce % 512:
            sizes.append(ce % 512)
        for nt in sizes:
            host_chunks.append((off, nt))
            off += nt

    def pack_w(e, h):
        lo, hi = h * F2, (h + 1) * F2
        w1p = np.ascontiguousarray(
            W1[e][:, lo:hi].reshape(KH, P, FT2, P).transpose(1, 2, 0, 3)
        )
        w2p = np.ascontiguousarray(
            W2[e][lo:hi, :].reshape(FT2, P, H).transpose(1, 0, 2)
        )
        return w1p.astype(np_dt), w2p.astype(np_dt)

    in_maps = [None] * N_CORES
    for pi, (a, b) in enumerate(pairs):
        xt = np.zeros((H, c_pad), np.float32)
        xt[:, : counts[a]] = x2d[rows[a]].T
        xt[:, ca_pad : ca_pad + counts[b]] = x2d[rows[b]].T
        xt = xt.reshape(KH, P, c_pad).astype(np_dt)
        xd = np.zeros((P, KH * c_pad), np_dt)
        for off, nt in host_chunks:
            xd[:, KH * off : KH * (off + nt)] = (
                xt[:, :, off : off + nt].transpose(1, 0, 2).reshape(P, KH * nt)
            )
        for h in range(2):
            w1a, w2a = pack_w(a, h)
            w1b, w2b = pack_w(b, h)
            m = {
                "xd": xd,
                "w1d": np.ascontiguousarray(np.concatenate([w1a, w1b], axis=1)),
                "w2d": np.ascontiguousarray(np.concatenate([w2a, w2b], axis=1)),
            }
            if use_b1:
                lo, hi = h * F2, (h + 1) * F2
                m["b1d"] = np.ascontiguousarray(
                    np.concatenate(
                        [
                            b1[a][lo:hi].reshape(FT2, P).T,
                            b1[b][lo:hi].reshape(FT2, P).T,
                        ],
                        axis=1,
                    )
                )
            in_maps[2 * pi + h] = m

    trace = os.environ.get("KERNEL_TRACE", "") == "1"
    res = run_bass_kernel_spmd(
        nc,
        in_maps,
        core_ids=list(range(N_CORES)),
        trace=trace,
        trace_cores=[0] if trace else None,
    )
    LAST_RESULT = res

    out = np.zeros((T, H), np.float32)
    for pi, (a, b) in enumerate(pairs):
        y = res.results[2 * pi]["yd"] + res.results[2 * pi + 1]["yd"]
        y = y.transpose(1, 0, 2).reshape(c_pad, H)
        out[rows[a]] += gval[a][:, None] * (y[: counts[a]] + b2[a][None, :])
        out[rows[b]] += gval[b][:, None] * (
            y[ca_pad : ca_pad + counts[b]] + b2[b][None, :]
        )

    return out.reshape(B, S, H)


# revision 28
# speedup vs baseline: 1.0005x; 1.0005x over previous
"""MoE FFN (top-2 of 8 experts) Trainium2 kernel, pair/F-split variant.

Same host-side routing as the resident-weight kernel, but load-balanced:
experts are sorted by token count and paired heavy-with-light; pair p is
assigned to cores (2p, 2p+1).  Each core of the pair holds HALF the F
dimension of BOTH experts' weights (same 16.8 MB bf16 SBUF footprint as
one full expert) and processes ALL of the pair's tokens at half F:

    per-core slots = ca_pad + cb_pad  (~4352 half-F token slots)
                   = ~2176 full-token equivalents vs 2304 for pure
                     expert-parallel -> ~6% less PE work.

GEMM2 contracts only the local F half, so each token's output is a
partial; the host sums the two cores' partials ("combine" still on
host).  Everything else (bf16, resident weights, chunk-major x, lag-1
GEMM2 pipeline, graded weight-slice streaming) matches the resident
kernel.
"""

import os
import sys
import numpy as np

for _p in ("/opt/trn_rl_repo", "/root/.axon_site/_ro/trn_rl_repo"):
    if _p not in sys.path and os.path.isdir(_p):
        sys.path.append(_p)

import concourse.bacc as bacc  # noqa: E402
import concourse.tile as tile  # noqa: E402
from concourse import mybir  # noqa: E402
from concourse.bass_utils import run_bass_kernel_spmd  # noqa: E402

# Problem shapes (hardcoded per spec)
B, S, H, F, E = 4, 2048, 1024, 4096, 8
T = B * S
TOP_K = 2
N_CORES = 8
P = 128
KH = H // P          # 8   H-contraction subtiles
F2 = F // 2          # 2048 F per core (half of one expert)
FT2 = F2 // P        # 16  f-tiles per expert half
CHUNK = 256          # tokens per GEMM1 chunk

F32 = mybir.dt.float32
BF16 = mybir.dt.bfloat16

_CACHE: dict = {}
LAST_RESULT = None  # BassKernelResults of the most recent run (for test.py)


def _build(ca: int, cb: int, use_b1: bool, mm_dt):
    nc = bacc.Bacc(
        "TRN2",
        target_bir_lowering=False,
        debug=False,
        enable_asserts=False,
        num_devices=N_CORES,
    )

    # ca/cb are EXACT max token counts (ceil-8); GEMM1 runs only the real
    # tokens while GEMM2/yd cover ceil-128 rows (t-tile granularity). The
    # few G2 rows beyond the exact count read stale hq data; their outputs
    # are padding rows the host discards, and a matmul's token columns are
    # independent so no real row is contaminated.
    ca_pad = -(-ca // P) * P
    cb_pad = -(-cb // P) * P
    c_pad = ca_pad + cb_pad
    NF = 2 * FT2  # 32 f-tiles total (16 per expert)
    # mixed 512/tail token chunks (512 keeps GEMM1 instruction count low)
    chunks = []  # (token_offset, nt, f0)
    for base, ce, f0 in ((0, ca, 0), (ca_pad, cb, FT2)):
        off = base
        sizes = [512] * (ce // 512)
        if ce % 512:
            sizes.append(ce % 512)
        for nt in sizes:
            chunks.append((off, nt, f0))
            off += nt
    n_chunks = len(chunks)

    # flat x layout: each chunk is a contiguous [KH, nt] block per partition
    xd = nc.dram_tensor(
        "xd", [P, KH * c_pad], mm_dt, kind="ExternalInput"
    ).ap()
    w1d = nc.dram_tensor("w1d", [P, NF, KH, P], mm_dt, kind="ExternalInput").ap()
    w2d = nc.dram_tensor("w2d", [P, NF, H], mm_dt, kind="ExternalInput").ap()
    if use_b1:
        b1d = nc.dram_tensor("b1d", [P, NF], F32, kind="ExternalInput").ap()
    yd = nc.dram_tensor("yd", [P, c_pad // P, H], F32, kind="ExternalOutput").ap()

    gelu = mybir.ActivationFunctionType.Gelu_apprx_tanh

    with tile.TileContext(nc) as tc:
        with (
            tc.tile_pool(name="w1p", bufs=1) as w1p,
            tc.tile_pool(name="w2p", bufs=1) as w2p,
            tc.tile_pool(name="xp", bufs=2) as xp,
            tc.tile_pool(name="hp", bufs=3) as hp,
            tc.tile_pool(name="op", bufs=4) as op,
            tc.tile_pool(name="bp", bufs=1) as bp,
            tc.tile_pool(name="ps1", bufs=3, space="PSUM") as ps1,
            tc.tile_pool(name="ps2", bufs=4, space="PSUM") as ps2,
            tc.tile_pool(name="wup", bufs=1) as wup,
            tc.tile_pool(name="psw", bufs=1, space="PSUM") as psw,
        ):
            # PE warmup: dummy matmuls on a zeroed scratch tile fill the
            # initial DMA-wait window so the HAM clock gate reaches 8/8
            # (2.4 GHz) before the first real matmul (~3.4us of sustained
            # PE activity required).
            wt = wup.tile([P, 512], mm_dt)
            nc.gpsimd.memset(wt[:], 0.0)
            wu_ps = psw.tile([P, 512], F32)
            for _ in range(26):
                nc.tensor.matmul(wu_ps[:], wt[:, :P], wt[:], start=True, stop=True)

            if use_b1:
                b1t = bp.tile([P, NF], F32)
                nc.sync.dma_start(b1t[:], b1d[:])

            # weight stream order: w1(expert a) graded, w2(a), w1(b), w2(b)
            w1 = w1p.tile([P, NF, KH, P], mm_dt)
            w2 = w2p.tile([P, NF, H], mm_dt)

            def stream_w(eh):  # eh = 0 (expert a) or 1 (expert b)
                base = eh * FT2
                slices = (1, 1, 2, 4, 8) if eh == 0 else (8, 8)
                i = base
                for n in slices:
                    nc.scalar.dma_start(w1[:, i : i + n], w1d[:, i : i + n])
                    i += n
                for i in range(base, base + FT2, 8):
                    nc.scalar.dma_start(w2[:, i : i + 8], w2d[:, i : i + 8])

            stream_w(0)
            stream_w(1)

            hqs = [None] * n_chunks

            def gemm1(ci):
                off, nt, f0 = chunks[ci]
                xt = xp.tile([P, KH * 512], mm_dt, tag="xt", name=f"xt_{ci}")
                nc.sync.dma_start(
                    xt[:, : KH * nt], xd[:, KH * off : KH * (off + nt)]
                )
                hq = hp.tile([P, FT2, 512], mm_dt, tag="hq", name=f"hq_{ci}")
                hqs[ci] = hq
                for fi in range(FT2):
                    f = f0 + fi
                    pt1 = ps1.tile([P, 512], F32, tag="pt1")
                    for k in range(KH):
                        nc.tensor.matmul(
                            pt1[:, :nt],
                            w1[:, f, k, :],
                            xt[:, k * nt : (k + 1) * nt],
                            start=(k == 0),
                            stop=(k == KH - 1),
                        )
                    bias = b1t[:, f : f + 1] if use_b1 else 0.0
                    nc.scalar.activation(hq[:, fi, :nt], pt1[:, :nt], gelu, bias=bias)

            def gemm2(ci):
                off, nt, f0 = chunks[ci]
                hq = hqs[ci]
                tcount = -(-nt // P)
                for t in range(tcount):
                    trow = off // P + t
                    last = ci == n_chunks - 1 and t == tcount - 1
                    pts = [
                        ps2.tile([P, 512], F32, tag="pt2", name=f"pt2_{hh}")
                        for hh in range(2)
                    ]
                    # last tile: finish hh halves sequentially so the first
                    # output DMA overlaps the second half's matmuls
                    hh_groups = [[0], [1]] if last else [[0, 1]]
                    for hhg in hh_groups:
                        for k2 in range(FT2):
                            for hh in hhg:
                                nc.tensor.matmul(
                                    pts[hh][:],
                                    hq[:, k2, t * P : (t + 1) * P],
                                    w2[:, f0 + k2, hh * 512 : (hh + 1) * 512],
                                    start=(k2 == 0),
                                    stop=(k2 == FT2 - 1),
                                )
                        if last:
                            hh = hhg[0]
                            ot = op.tile([P, 512], F32, tag="ot")
                            nc.vector.tensor_copy(ot[:], pts[hh][:])
                            nc.sync.dma_start(
                                yd[:, trow, hh * 512 : (hh + 1) * 512], ot[:]
                            )
                    if last:
                        hqs[ci] = None
                        return
                    for hh in range(2):
                        ot = op.tile([P, 512], F32, tag="ot")
                        nc.vector.tensor_copy(ot[:], pts[hh][:])
                        nc.sync.dma_start(
                            yd[:, trow, hh * 512 : (hh + 1) * 512], ot[:]
                        )
                hqs[ci] = None

            for ci in range(n_chunks):
                gemm1(ci)
                if ci >= 1:
                    gemm2(ci - 1)
            gemm2(n_chunks - 1)

    nc.compile()
    return nc


def _route(x2d, Wg):
    """Replicates reference router: softmax -> top-2 -> renormalize."""
    logits = x2d @ Wg  # [T, E] fp32
    m = logits.max(axis=-1, keepdims=True)
    p = np.exp(logits - m, dtype=np.float32)
    p /= p.sum(axis=-1, keepdims=True)
    # jax.lax.top_k: values descending, ties broken by lower index.
    order = np.argsort(-p, axis=-1, kind="stable")
    top_i = order[:, :TOP_K]  # [T, 2]
    top_p = np.take_along_axis(p, top_i, axis=-1)
    top_p = top_p / top_p.sum(axis=-1, keepdims=True)
    return top_i, top_p


def _pad8(c: int) -> int:
    return max(8, ((c + 7) // 8) * 8)


def kernel(x, Wg, W1, b1, W2, b2):
    global LAST_RESULT
    x = np.ascontiguousarray(np.asarray(x, dtype=np.float32))
    Wg = np.ascontiguousarray(np.asarray(Wg, dtype=np.float32))
    W1 = np.ascontiguousarray(np.asarray(W1, dtype=np.float32))
    b1 = np.ascontiguousarray(np.asarray(b1, dtype=np.float32))
    W2 = np.ascontiguousarray(np.asarray(W2, dtype=np.float32))
    b2 = np.ascontiguousarray(np.asarray(b2, dtype=np.float32))

    x2d = x.reshape(T, H)
    top_i, top_p = _route(x2d, Wg)

    rows = [None] * E
    gval = [None] * E
    for e in range(E):
        r, slot = np.nonzero(top_i == e)
        rows[e] = r
        gval[e] = top_p[r, slot]

    counts = np.array([len(r) for r in rows])
    order = np.argsort(-counts, kind="stable")
    pairs = [(int(order[i]), int(order[E - 1 - i])) for i in range(E // 2)]
    ca = _pad8(int(counts[[p[0] for p in pairs]].max()))
    cb = _pad8(int(counts[[p[1] for p in pairs]].max()))
    use_b1 = bool(np.any(b1))

    mm_dt = {
        "bf16": BF16,
        "fp32": F32,
    }[os.environ.get("KERNEL_MMDT", "bf16")]
    key = (ca, cb, use_b1, str(mm_dt))
    if key not in _CACHE:
        _CACHE[key] = _build(ca, cb, use_b1, mm_dt)
    nc = _CACHE[key]

    np_dt = mybir.dt.np(mm_dt)
    ca_pad = -(-ca // P) * P
    cb_pad = -(-cb // P) * P
    c_pad = ca_pad + cb_pad
    host_chunks = []  # (token_offset, nt) mirroring _build
    for base, ce in ((0, ca), (ca_pad, cb)):
        off = base
        sizes = [512] * (ce // 512)
        if ce % 512:
            sizes.append(ce % 512)
        for nt in sizes:
            host_chunks.append((off, nt))
            off += nt

    def pack_w(e, h):
        lo, hi = h * F2, (h + 1) * F2
        w1p = np.ascontiguousarray(
            W1[e][:, lo:hi].reshape(KH, P, FT2, P).transpose(1, 2, 0, 3)
        )
        w2p = np.ascontiguousarray(
            W2[e][lo:hi, :].reshape(FT2, P, H).transpose(1, 0, 2)
        )
        return w1p.astype(np_dt), w2p.astype(np_dt)

    in_maps = [None] * N_CORES
    for pi, (a, b) in enumerate(pairs):
        xt = np.zeros((H, c_pad), np.float32)
        xt[:, : counts[a]] = x2d[rows[a]].T
        xt[:, ca_pad : ca_pad + counts[b]] = x2d[rows[b]].T
        xt = xt.reshape(KH, P, c_pad).astype(np_dt)
        xd = np.zeros((P, KH * c_pad), np_dt)
        for off, nt in host_chunks:
            xd[:, KH * off : KH * (off + nt)] = (
                xt[:, :, off : off + nt].transpose(1, 0, 2).reshape(P, KH * nt)
            )
        for h in range(2):
            w1a, w2a = pack_w(a, h)
            w1b, w2b = pack_w(b, h)
            m = {
                "xd": xd,
                "w1d": np.ascontiguousarray(np.concatenate([w1a, w1b], axis=1)),
                "w2d": np.ascontiguousarray(np.concatenate([w2a, w2b], axis=1)),
            }
            if use_b1:
                lo, hi = h * F2, (h + 1) * F2
                m["b1d"] = np.ascontiguousarray(
                    np.concatenate(
                        [
                            b1[a][lo:hi].reshape(FT2, P).T,
                            b1[b][lo:hi].reshape(FT2, P).T,
                        ],
                        axis=1,
                    )
                )
            in_maps[2 * pi + h] = m

    trace = os.environ.get("KERNEL_TRACE", "") == "1"
    res = run_bass_kernel_spmd(
        nc,
        in_maps,
        core_ids=list(range(N_CORES)),
        trace=trace,
        trace_cores=[0] if trace else None,
    )
    LAST_RESULT = res

    out = np.zeros((T, H), np.float32)
    for pi, (a, b) in enumerate(pairs):
        y = res.results[2 * pi]["yd"] + res.results[2 * pi + 1]["yd"]
        y = y.transpose(1, 0, 2).reshape(c_pad, H)
        out[rows[a]] += gval[a][:, None] * (y[: counts[a]] + b2[a][None, :])
        out[rows[b]] += gval[b][:, None] * (
            y[ca_pad : ca_pad + counts[b]] + b2[b][None, :]
        )

    return out.reshape(B, S, H)


# revision 30
# speedup vs baseline: 1.0099x; 1.0095x over previous
"""MoE FFN (top-2 of 8 experts) Trainium2 kernel, quad/F-quarter variant.

Experts are sorted by routed-token count and split into two groups of 4
by interleaving (1st,3rd,5th,7th | 2nd,4th,6th,8th), which minimizes the
slot-wise maxima the uniform SPMD program must cover.  Group A runs on
cores 0-3, group B on cores 4-7; core (grp, h) holds QUARTER h of the F
dimension of all 4 of its group's experts (same 16.8 MB bf16 resident
footprint) and processes all the group's tokens at quarter F.

Per-core work: GEMM1 = sum of exact slot capacities (~8288 quarter-F
token rows), GEMM2 = sum of ceil-128 capacities (~8576 rows) -- a few
percent less than the pair/F-half scheme.  Each token's output is a
4-way partial; the host sums the 4 cores' partials and combines.

Everything else matches the pair kernel: bf16, resident weights, flat
chunk-major x, graded weight-slice streaming, GEMM2-lags-GEMM1 pipeline,
PE warmup matmuls, exact GEMM1 tails with ceil-128 GEMM2 t-tiles.
"""

import os
import sys
import numpy as np

for _p in ("/opt/trn_rl_repo", "/root/.axon_site/_ro/trn_rl_repo"):
    if _p not in sys.path and os.path.isdir(_p):
        sys.path.append(_p)

import concourse.bacc as bacc  # noqa: E402
import concourse.tile as tile  # noqa: E402
from concourse import mybir  # noqa: E402
from concourse.bass_utils import run_bass_kernel_spmd  # noqa: E402

# Problem shapes (hardcoded per spec)
B, S, H, F, E = 4, 2048, 1024, 4096, 8
T = B * S
TOP_K = 2
N_CORES = 8
P = 128
KH = H // P          # 8   H-contraction subtiles
NSLOT = 4            # experts per core
FQ = F // NSLOT      # 1024 F per core per expert (quarter)
FTQ = FQ // P        # 8   f-tiles per expert quarter
NF = NSLOT * FTQ     # 32  f-tiles total

F32 = mybir.dt.float32
BF16 = mybir.dt.bfloat16

_CACHE: dict = {}
LAST_RESULT = None  # BassKernelResults of the most recent run (for test.py)


def _chunk_sizes(ce: int):
    """512-chunks with an exact tail.  Tails <128 (below the LDWEIGHTS
    shadow) are rebalanced as 512+rem -> (256, 256+rem); every non-final
    chunk stays a multiple of 128 so t-tile rows (off // 128) align."""
    sizes = [512] * (ce // 512)
    rem = ce % 512
    if rem:
        if rem < 128 and sizes:
            sizes.pop()
            sizes += [256, 256 + rem]
        else:
            sizes.append(rem)
    return sizes


def _build(caps: tuple, use_b1: bool, mm_dt):
    nc = bacc.Bacc(
        "TRN2",
        target_bir_lowering=False,
        debug=False,
        enable_asserts=False,
        num_devices=N_CORES,
    )

    # caps: EXACT (ceil-8) token capacity per slot. GEMM1 runs exact rows;
    # GEMM2/yd cover ceil-128 rows per slot (stale-hq reads beyond the
    # exact count only affect discarded padding rows).
    pads = [-(-c // P) * P for c in caps]
    bases = [sum(pads[:s]) for s in range(NSLOT)]
    c_pad = sum(pads)

    chunks = []  # (row_offset, nt, f0)
    for s in range(NSLOT):
        off = bases[s]
        for nt in _chunk_sizes(caps[s]):
            chunks.append((off, nt, s * FTQ))
            off += nt
    n_chunks = len(chunks)

    xd = nc.dram_tensor("xd", [P, KH * c_pad], mm_dt, kind="ExternalInput").ap()
    w1d = nc.dram_tensor("w1d", [P, NF, KH, P], mm_dt, kind="ExternalInput").ap()
    w2d = nc.dram_tensor("w2d", [P, NF, H], mm_dt, kind="ExternalInput").ap()
    if use_b1:
        b1d = nc.dram_tensor("b1d", [P, NF], F32, kind="ExternalInput").ap()
    yd = nc.dram_tensor("yd", [P, c_pad // P, H], F32, kind="ExternalOutput").ap()

    gelu = mybir.ActivationFunctionType.Gelu_apprx_tanh

    with tile.TileContext(nc) as tc:
        with (
            tc.tile_pool(name="w1p", bufs=1) as w1p,
            tc.tile_pool(name="w2p", bufs=1) as w2p,
            tc.tile_pool(name="xp", bufs=2) as xp,
            tc.tile_pool(name="hp", bufs=3) as hp,
            tc.tile_pool(name="op", bufs=4) as op,
            tc.tile_pool(name="bp", bufs=1) as bp,
            tc.tile_pool(name="ps1", bufs=3, space="PSUM") as ps1,
            tc.tile_pool(name="ps2", bufs=4, space="PSUM") as ps2,
            tc.tile_pool(name="wup", bufs=1) as wup,
            tc.tile_pool(name="psw", bufs=1, space="PSUM") as psw,
        ):
            # PE warmup: dummy matmuls fill the initial DMA-wait window so
            # the HAM clock gate reaches 8/8 before the first real matmul.
            wt = wup.tile([P, 512], mm_dt)
            nc.gpsimd.memset(wt[:], 0.0)
            wu_ps = psw.tile([P, 512], F32)
            for _ in range(26):
                nc.tensor.matmul(wu_ps[:], wt[:, :P], wt[:], start=True, stop=True)

            if use_b1:
                b1t = bp.tile([P, NF], F32)
                nc.sync.dma_start(b1t[:], b1d[:])

            # per-slot weight stream: w1(slot) then w2(slot); first slot's
            # w1 graded so GEMM1 can start on the first f-tile
            w1 = w1p.tile([P, NF, KH, P], mm_dt)
            w2 = w2p.tile([P, NF, H], mm_dt)
            for s in range(NSLOT):
                base = s * FTQ
                slices = (1, 1, 2, 4) if s == 0 else (8,)
                i = base
                for n in slices:
                    nc.scalar.dma_start(w1[:, i : i + n], w1d[:, i : i + n])
                    i += n
                if s == 0:
                    nc.scalar.dma_start(w2[:, base : base + 4], w2d[:, base : base + 4])
                    nc.scalar.dma_start(
                        w2[:, base + 4 : base + 8], w2d[:, base + 4 : base + 8]
                    )
                else:
                    nc.scalar.dma_start(
                        w2[:, base : base + FTQ], w2d[:, base : base + FTQ]
                    )

            hqs = [None] * n_chunks

            def gemm1(ci):
                off, nt, f0 = chunks[ci]
                xt = xp.tile([P, KH * 512], mm_dt, tag="xt", name=f"xt_{ci}")
                nc.sync.dma_start(
                    xt[:, : KH * nt], xd[:, KH * off : KH * (off + nt)]
                )
                hq = hp.tile([P, FTQ, 512], mm_dt, tag="hq", name=f"hq_{ci}")
                hqs[ci] = hq
                for fi in range(FTQ):
                    f = f0 + fi
                    pt1 = ps1.tile([P, 512], F32, tag="pt1")
                    for k in range(KH):
                        nc.tensor.matmul(
                            pt1[:, :nt],
                            w1[:, f, k, :],
                            xt[:, k * nt : (k + 1) * nt],
                            start=(k == 0),
                            stop=(k == KH - 1),
                        )
                    bias = b1t[:, f : f + 1] if use_b1 else 0.0
                    nc.scalar.activation(hq[:, fi, :nt], pt1[:, :nt], gelu, bias=bias)

            def gemm2(ci):
                off, nt, f0 = chunks[ci]
                hq = hqs[ci]
                tcount = -(-nt // P)
                for t in range(tcount):
                    trow = off // P + t
                    last = ci == n_chunks - 1 and t == tcount - 1
                    pts = [
                        ps2.tile([P, 512], F32, tag="pt2", name=f"pt2_{hh}")
                        for hh in range(2)
                    ]
                    # last tile: finish hh halves sequentially so the first
                    # output DMA overlaps the second half's matmuls
                    hh_groups = [[0], [1]] if last else [[0, 1]]
                    for hhg in hh_groups:
                        for k2 in range(FTQ):
                            for hh in hhg:
                                nc.tensor.matmul(
                                    pts[hh][:],
                                    hq[:, k2, t * P : (t + 1) * P],
                                    w2[:, f0 + k2, hh * 512 : (hh + 1) * 512],
                                    start=(k2 == 0),
                                    stop=(k2 == FTQ - 1),
                                )
                        if last:
                            hh = hhg[0]
                            ot = op.tile([P, 512], F32, tag="ot")
                            nc.vector.tensor_copy(ot[:], pts[hh][:])
                            nc.sync.dma_start(
                                yd[:, trow, hh * 512 : (hh + 1) * 512], ot[:]
                            )
                    if last:
                        hqs[ci] = None
                        return
                    for hh in range(2):
                        ot = op.tile([P, 512], F32, tag="ot")
                        nc.vector.tensor_copy(ot[:], pts[hh][:])
                        nc.sync.dma_start(
                            yd[:, trow, hh * 512 : (hh + 1) * 512], ot[:]
                        )
                hqs[ci] = None

            for ci in range(n_chunks):
                gemm1(ci)
                if ci >= 1:
                    gemm2(ci - 1)
            gemm2(n_chunks - 1)

    nc.compile()
    return nc


def _route(x2d, Wg):
    """Replicates reference router: softmax -> top-2 -> renormalize."""
    logits = x2d @ Wg  # [T, E] fp32
    m = logits.max(axis=-1, keepdims=True)
    p = np.exp(logits - m, dtype=np.float32)
    p /= p.sum(axis=-1, keepdims=True)
    # jax.lax.top_k: values descending, ties broken by lower index.
    order = np.argsort(-p, axis=-1, kind="stable")
    top_i = order[:, :TOP_K]  # [T, 2]
    top_p = np.take_along_axis(p, top_i, axis=-1)
    top_p = top_p / top_p.sum(axis=-1, keepdims=True)
    return top_i, top_p


def _pad8(c: int) -> int:
    return max(8, ((c + 7) // 8) * 8)


def kernel(x, Wg, W1, b1, W2, b2):
    global LAST_RESULT
    x = np.ascontiguousarray(np.asarray(x, dtype=np.float32))
    Wg = np.ascontiguousarray(np.asarray(Wg, dtype=np.float32))
    W1 = np.ascontiguousarray(np.asarray(W1, dtype=np.float32))
    b1 = np.ascontiguousarray(np.asarray(b1, dtype=np.float32))
    W2 = np.ascontiguousarray(np.asarray(W2, dtype=np.float32))
    b2 = np.ascontiguousarray(np.asarray(b2, dtype=np.float32))

    x2d = x.reshape(T, H)
    top_i, top_p = _route(x2d, Wg)

    rows = [None] * E
    gval = [None] * E
    for e in range(E):
        r, slot = np.nonzero(top_i == e)
        rows[e] = r
        gval[e] = top_p[r, slot]

    counts = np.array([len(r) for r in rows])
    order = np.argsort(-counts, kind="stable")
    groups = [[int(order[2 * s + g]) for s in range(NSLOT)] for g in range(2)]
    caps = tuple(
        _pad8(int(max(counts[groups[0][s]], counts[groups[1][s]])))
        for s in range(NSLOT)
    )
    use_b1 = bool(np.any(b1))

    mm_dt = {
        "bf16": BF16,
        "fp32": F32,
    }[os.environ.get("KERNEL_MMDT", "bf16")]
    key = (caps, use_b1, str(mm_dt))
    if key not in _CACHE:
        _CACHE[key] = _build(caps, use_b1, mm_dt)
    nc = _CACHE[key]

    np_dt = mybir.dt.np(mm_dt)
    pads = [-(-c // P) * P for c in caps]
    bases = [sum(pads[:s]) for s in range(NSLOT)]
    c_pad = sum(pads)
    host_chunks = []  # (row_offset, nt)
    for s in range(NSLOT):
        off = bases[s]
        for nt in _chunk_sizes(caps[s]):
            host_chunks.append((off, nt))
            off += nt

    def pack_w(e, h):
        lo, hi = h * FQ, (h + 1) * FQ
        w1p_ = np.ascontiguousarray(
            W1[e][:, lo:hi].reshape(KH, P, FTQ, P).transpose(1, 2, 0, 3)
        )
        w2p_ = np.ascontiguousarray(
            W2[e][lo:hi, :].reshape(FTQ, P, H).transpose(1, 0, 2)
        )
        return w1p_.astype(np_dt), w2p_.astype(np_dt)

    in_maps = [None] * N_CORES
    for g in range(2):
        xt = np.zeros((H, c_pad), np.float32)
        for s in range(NSLOT):
            e = groups[g][s]
            xt[:, bases[s] : bases[s] + counts[e]] = x2d[rows[e]].T
        xt = xt.reshape(KH, P, c_pad).astype(np_dt)
        xd = np.zeros((P, KH * c_pad), np_dt)
        for off, nt in host_chunks:
            xd[:, KH * off : KH * (off + nt)] = (
                xt[:, :, off : off + nt].transpose(1, 0, 2).reshape(P, KH * nt)
            )
        for h in range(NSLOT):
            w1s, w2s = [], []
            for s in range(NSLOT):
                a, b_ = pack_w(groups[g][s], h)
                w1s.append(a)
                w2s.append(b_)
            m = {
                "xd": xd,
                "w1d": np.ascontiguousarray(np.concatenate(w1s, axis=1)),
                "w2d": np.ascontiguousarray(np.concatenate(w2s, axis=1)),
            }
            if use_b1:
                lo, hi = h * FQ, (h + 1) * FQ
                m["b1d"] = np.ascontiguousarray(
                    np.concatenate(
                        [
                            b1[groups[g][s]][lo:hi].reshape(FTQ, P).T
                            for s in range(NSLOT)
                        ],
                        axis=1,
                    )
                )
            in_maps[g * NSLOT + h] = m

    trace = os.environ.get("KERNEL_TRACE", "") == "1"
    res = run_bass_kernel_spmd(
        nc,
        in_maps,
        core_ids=list(range(N_CORES)),
        trace=trace,
        trace_cores=[0] if trace else None,
    )
    LAST_RESULT = res

    out = np.zeros((T, H), np.float32)
    for g in range(2):
        y = res.results[g * NSLOT]["yd"].copy()
        for h in range(1, NSLOT):
            y += res.results[g * NSLOT + h]["yd"]
        y = y.transpose(1, 0, 2).reshape(c_pad, H)
        for s in range(NSLOT):
            e = groups[g][s]
            out[rows[e]] += gval[e][:, None] * (
                y[bases[s] : bases[s] + counts[e]] + b2[e][None, :]
            )

    return out.reshape(B, S, H)


# revision 31
# speedup vs baseline: 1.0105x; 1.0005x over previous
"""MoE FFN (top-2 of 8 experts) Trainium2 kernel, quad/F-quarter variant.

Experts are sorted by routed-token count and split into two groups of 4
by interleaving (1st,3rd,5th,7th | 2nd,4th,6th,8th), which minimizes the
slot-wise maxima the uniform SPMD program must cover.  Group A runs on
cores 0-3, group B on cores 4-7; core (grp, h) holds QUARTER h of the F
dimension of all 4 of its group's experts (same 16.8 MB bf16 resident
footprint) and processes all the group's tokens at quarter F.

Per-core work: GEMM1 = sum of exact slot capacities (~8288 quarter-F
token rows), GEMM2 = sum of ceil-128 capacities (~8576 rows) -- a few
percent less than the pair/F-half scheme.  Each token's output is a
4-way partial; the host sums the 4 cores' partials and combines.

Everything else matches the pair kernel: bf16, resident weights, flat
chunk-major x, graded weight-slice streaming, GEMM2-lags-GEMM1 pipeline,
PE warmup matmuls, exact GEMM1 tails with ceil-128 GEMM2 t-tiles.
"""

import os
import sys
import numpy as np

for _p in ("/opt/trn_rl_repo", "/root/.axon_site/_ro/trn_rl_repo"):
    if _p not in sys.path and os.path.isdir(_p):
        sys.path.append(_p)

import concourse.bacc as bacc  # noqa: E402
import concourse.tile as tile  # noqa: E402
from concourse import mybir  # noqa: E402
from concourse.bass_utils import run_bass_kernel_spmd  # noqa: E402

# Problem shapes (hardcoded per spec)
B, S, H, F, E = 4, 2048, 1024, 4096, 8
T = B * S
TOP_K = 2
N_CORES = 8
P = 128
KH = H // P          # 8   H-contraction subtiles
NSLOT = 4            # experts per core
FQ = F // NSLOT      # 1024 F per core per expert (quarter)
FTQ = FQ // P        # 8   f-tiles per expert quarter
NF = NSLOT * FTQ     # 32  f-tiles total

F32 = mybir.dt.float32
BF16 = mybir.dt.bfloat16

_CACHE: dict = {}
LAST_RESULT = None  # BassKernelResults of the most recent run (for test.py)


def _chunk_sizes(ce: int):
    """512-chunks with an exact tail.  Tails <128 (below the LDWEIGHTS
    shadow) are rebalanced as 512+rem -> (256, 256+rem); every non-final
    chunk stays a multiple of 128 so t-tile rows (off // 128) align."""
    sizes = [512] * (ce // 512)
    rem = ce % 512
    if rem:
        if rem < 128 and sizes:
            sizes.pop()
            sizes += [256, 256 + rem]
        else:
            sizes.append(rem)
    return sizes


def _build(caps: tuple, use_b1: bool, mm_dt):
    nc = bacc.Bacc(
        "TRN2",
        target_bir_lowering=False,
        debug=False,
        enable_asserts=False,
        num_devices=N_CORES,
    )

    # caps: EXACT (ceil-8) token capacity per slot. GEMM1 runs exact rows;
    # GEMM2/yd cover ceil-128 rows per slot (stale-hq reads beyond the
    # exact count only affect discarded padding rows).
    pads = [-(-c // P) * P for c in caps]
    bases = [sum(pads[:s]) for s in range(NSLOT)]
    c_pad = sum(pads)

    chunks = []  # (row_offset, nt, f0)
    for s in range(NSLOT):
        off = bases[s]
        for nt in _chunk_sizes(caps[s]):
            chunks.append((off, nt, s * FTQ))
            off += nt
    n_chunks = len(chunks)

    xd = nc.dram_tensor("xd", [P, KH * c_pad], mm_dt, kind="ExternalInput").ap()
    w1d = nc.dram_tensor("w1d", [P, NF, KH, P], mm_dt, kind="ExternalInput").ap()
    w2d = nc.dram_tensor("w2d", [P, NF, H], mm_dt, kind="ExternalInput").ap()
    if use_b1:
        b1d = nc.dram_tensor("b1d", [P, NF], F32, kind="ExternalInput").ap()
    yd = nc.dram_tensor("yd", [P, c_pad // P, H], F32, kind="ExternalOutput").ap()

    gelu = mybir.ActivationFunctionType.Gelu_apprx_tanh

    with tile.TileContext(nc) as tc:
        with (
            tc.tile_pool(name="w1p", bufs=1) as w1p,
            tc.tile_pool(name="w2p", bufs=1) as w2p,
            tc.tile_pool(name="xp", bufs=2) as xp,
            tc.tile_pool(name="hp", bufs=3) as hp,
            tc.tile_pool(name="op", bufs=4) as op,
            tc.tile_pool(name="bp", bufs=1) as bp,
            tc.tile_pool(name="ps1", bufs=3, space="PSUM") as ps1,
            tc.tile_pool(name="ps2", bufs=4, space="PSUM") as ps2,
            tc.tile_pool(name="wup", bufs=1) as wup,
            tc.tile_pool(name="psw", bufs=1, space="PSUM") as psw,
        ):
            # PE warmup: dummy matmuls cover the HAM activity window (~3.4us
            # of sustained PE busy at the cold 1.2GHz clock) so the clock
            # gate reaches 8/8 just as the first weights land; more than
            # that queues ahead of ready real work and wastes PE time.
            wt = wup.tile([P, 512], mm_dt)
            nc.gpsimd.memset(wt[:], 0.0)
            wu_ps = psw.tile([P, 512], F32)
            for _ in range(10):
                nc.tensor.matmul(wu_ps[:], wt[:, :P], wt[:], start=True, stop=True)

            if use_b1:
                b1t = bp.tile([P, NF], F32)
                nc.sync.dma_start(b1t[:], b1d[:])

            # per-slot weight stream: w1(slot) then w2(slot); first slot's
            # w1 graded so GEMM1 can start on the first f-tile
            w1 = w1p.tile([P, NF, KH, P], mm_dt)
            w2 = w2p.tile([P, NF, H], mm_dt)
            for s in range(NSLOT):
                base = s * FTQ
                slices = (1, 1, 2, 4) if s == 0 else (8,)
                i = base
                for n in slices:
                    nc.scalar.dma_start(w1[:, i : i + n], w1d[:, i : i + n])
                    i += n
                if s == 0:
                    nc.scalar.dma_start(w2[:, base : base + 4], w2d[:, base : base + 4])
                    nc.scalar.dma_start(
                        w2[:, base + 4 : base + 8], w2d[:, base + 4 : base + 8]
                    )
                else:
                    nc.scalar.dma_start(
                        w2[:, base : base + FTQ], w2d[:, base : base + FTQ]
                    )

            hqs = [None] * n_chunks

            def gemm1(ci):
                off, nt, f0 = chunks[ci]
                xt = xp.tile([P, KH * 512], mm_dt, tag="xt", name=f"xt_{ci}")
                nc.sync.dma_start(
                    xt[:, : KH * nt], xd[:, KH * off : KH * (off + nt)]
                )
                hq = hp.tile([P, FTQ, 512], mm_dt, tag="hq", name=f"hq_{ci}")
                hqs[ci] = hq
                for fi in range(FTQ):
                    f = f0 + fi
                    pt1 = ps1.tile([P, 512], F32, tag="pt1")
                    for k in range(KH):
                        nc.tensor.matmul(
                            pt1[:, :nt],
                            w1[:, f, k, :],
                            xt[:, k * nt : (k + 1) * nt],
                            start=(k == 0),
                            stop=(k == KH - 1),
                        )
                    bias = b1t[:, f : f + 1] if use_b1 else 0.0
                    nc.scalar.activation(hq[:, fi, :nt], pt1[:, :nt], gelu, bias=bias)

            def gemm2(ci):
                off, nt, f0 = chunks[ci]
                hq = hqs[ci]
                tcount = -(-nt // P)
                for t in range(tcount):
                    trow = off // P + t
                    last = ci == n_chunks - 1 and t == tcount - 1
                    pts = [
                        ps2.tile([P, 512], F32, tag="pt2", name=f"pt2_{hh}")
                        for hh in range(2)
                    ]
                    # last tile: finish hh halves sequentially so the first
                    # output DMA overlaps the second half's matmuls
                    hh_groups = [[0], [1]] if last else [[0, 1]]
                    for hhg in hh_groups:
                        for k2 in range(FTQ):
                            for hh in hhg:
                                nc.tensor.matmul(
                                    pts[hh][:],
                                    hq[:, k2, t * P : (t + 1) * P],
                                    w2[:, f0 + k2, hh * 512 : (hh + 1) * 512],
                                    start=(k2 == 0),
                                    stop=(k2 == FTQ - 1),
                                )
                        if last:
                            hh = hhg[0]
                            ot = op.tile([P, 512], F32, tag="ot")
                            nc.vector.tensor_copy(ot[:], pts[hh][:])
                            nc.sync.dma_start(
                                yd[:, trow, hh * 512 : (hh + 1) * 512], ot[:]
                            )
                    if last:
                        hqs[ci] = None
                        return
                    for hh in range(2):
                        ot = op.tile([P, 512], F32, tag="ot")
                        nc.vector.tensor_copy(ot[:], pts[hh][:])
                        nc.sync.dma_start(
                            yd[:, trow, hh * 512 : (hh + 1) * 512], ot[:]
                        )
                hqs[ci] = None

            for ci in range(n_chunks):
                gemm1(ci)
                if ci >= 1:
                    gemm2(ci - 1)
            gemm2(n_chunks - 1)

    nc.compile()
    return nc


def _route(x2d, Wg):
    """Replicates reference router: softmax -> top-2 -> renormalize."""
    logits = x2d @ Wg  # [T, E] fp32
    m = logits.max(axis=-1, keepdims=True)
    p = np.exp(logits - m, dtype=np.float32)
    p /= p.sum(axis=-1, keepdims=True)
    # jax.lax.top_k: values descending, ties broken by lower index.
    order = np.argsort(-p, axis=-1, kind="stable")
    top_i = order[:, :TOP_K]  # [T, 2]
    top_p = np.take_along_axis(p, top_i, axis=-1)
    top_p = top_p / top_p.sum(axis=-1, keepdims=True)
    return top_i, top_p


def _pad8(c: int) -> int:
    return max(8, ((c + 7) // 8) * 8)


def kernel(x, Wg, W1, b1, W2, b2):
    global LAST_RESULT
    x = np.ascontiguousarray(np.asarray(x, dtype=np.float32))
    Wg = np.ascontiguousarray(np.asarray(Wg, dtype=np.float32))
    W1 = np.ascontiguousarray(np.asarray(W1, dtype=np.float32))
    b1 = np.ascontiguousarray(np.asarray(b1, dtype=np.float32))
    W2 = np.ascontiguousarray(np.asarray(W2, dtype=np.float32))
    b2 = np.ascontiguousarray(np.asarray(b2, dtype=np.float32))

    x2d = x.reshape(T, H)
    top_i, top_p = _route(x2d, Wg)

    rows = [None] * E
    gval = [None] * E
    for e in range(E):
        r, slot = np.nonzero(top_i == e)
        rows[e] = r
        gval[e] = top_p[r, slot]

    counts = np.array([len(r) for r in rows])
    order = np.argsort(-counts, kind="stable")
    groups = [[int(order[2 * s + g]) for s in range(NSLOT)] for g in range(2)]
    caps = tuple(
        _pad8(int(max(counts[groups[0][s]], counts[groups[1][s]])))
        for s in range(NSLOT)
    )
    use_b1 = bool(np.any(b1))

    mm_dt = {
        "bf16": BF16,
        "fp32": F32,
    }[os.environ.get("KERNEL_MMDT", "bf16")]
    key = (caps, use_b1, str(mm_dt))
    if key not in _CACHE:
        _CACHE[key] = _build(caps, use_b1, mm_dt)
    nc = _CACHE[key]

    np_dt = mybir.dt.np(mm_dt)
    pads = [-(-c // P) * P for c in caps]
    bases = [sum(pads[:s]) for s in range(NSLOT)]
    c_pad = sum(pads)
    host_chunks = []  # (row_offset, nt)
    for s in range(NSLOT):
        off = bases[s]
        for nt in _chunk_sizes(caps[s]):
            host_chunks.append((off, nt))
            off += nt

    def pack_w(e, h):
        lo, hi = h * FQ, (h + 1) * FQ
        w1p_ = np.ascontiguousarray(
            W1[e][:, lo:hi].reshape(KH, P, FTQ, P).transpose(1, 2, 0, 3)
        )
        w2p_ = np.ascontiguousarray(
            W2[e][lo:hi, :].reshape(FTQ, P, H).transpose(1, 0, 2)
        )
        return w1p_.astype(np_dt), w2p_.astype(np_dt)

    in_maps = [None] * N_CORES
    for g in range(2):
        xt = np.zeros((H, c_pad), np.float32)
        for s in range(NSLOT):
            e = groups[g][s]
            xt[:, bases[s] : bases[s] + counts[e]] = x2d[rows[e]].T
        xt = xt.reshape(KH, P, c_pad).astype(np_dt)
        xd = np.zeros((P, KH * c_pad), np_dt)
        for off, nt in host_chunks:
            xd[:, KH * off : KH * (off + nt)] = (
                xt[:, :, off : off + nt].transpose(1, 0, 2).reshape(P, KH * nt)
            )
        for h in range(NSLOT):
            w1s, w2s = [], []
            for s in range(NSLOT):
                a, b_ = pack_w(groups[g][s], h)
                w1s.append(a)
                w2s.append(b_)
            m = {
                "xd": xd,
                "w1d": np.ascontiguousarray(np.concatenate(w1s, axis=1)),
                "w2d": np.ascontiguousarray(np.concatenate(w2s, axis=1)),
            }
            if use_b1:
                lo, hi = h * FQ, (h + 1) * FQ
                m["b1d"] = np.ascontiguousarray(
                    np.concatenate(
                        [
                            b1[groups[g][s]][lo:hi].reshape(FTQ, P).T
                            for s in range(NSLOT)
                        ],
                        axis=1,
                    )
                )
            in_maps[g * NSLOT + h] = m

    trace = os.environ.get("KERNEL_TRACE", "") == "1"
    res = run_bass_kernel_spmd(
        nc,
        in_maps,
        core_ids=list(range(N_CORES)),
        trace=trace,
        trace_cores=[0] if trace else None,
    )
    LAST_RESULT = res

    out = np.zeros((T, H), np.float32)
    for g in range(2):
        y = res.results[g * NSLOT]["yd"].copy()
        for h in range(1, NSLOT):
            y += res.results[g * NSLOT + h]["yd"]
        y = y.transpose(1, 0, 2).reshape(c_pad, H)
        for s in range(NSLOT):
            e = groups[g][s]
            out[rows[e]] += gval[e][:, None] * (
                y[bases[s] : bases[s] + counts[e]] + b2[e][None, :]
            )

    return out.reshape(B, S, H)
